# revision 2
# baseline (speedup 1.0000x reference)
"""Trainium2 Bass kernel for causal multi-head attention with RoPE — v2.

Sharding: tensor-parallel over heads (2 heads/core); the host sums the 8
partial outputs and folds the (zero / exactly-foldable) bias terms.

Techniques vs v1 (which was fp16 matmuls throughout):
- All four projections run as 3-term scaled-residual fp8e4 DoubleRow
  matmuls (K=256 per instruction): with x ~ x8 + dx/16 and
  64*w ~ w8 + dw/16,
      1024*(x@w) = x8@(16*w8) + (16*dx)@w8 + x8@dw16
  exact to second order; the final psum->SBUF copy applies 1/1024.
  Weights are pre-scaled by 64 so fp8 quantization avoids subnormals;
  residuals are pre-scaled by 16 for the same reason.
- The attention-value matmul swaps stationary/moving: lhsT = probs tile
  [k,q], rhs = v block [k, 129] whose column 128 holds 1/16, producing
  psum [q, hd | denom/16].  The softmax denominator comes out of the AV
  matmul itself (no separate ones-matmul), and normalization folds into
  the psum->SBUF ACT copy via a per-partition scale AP (= 16/denom,
  which simultaneously provides the x16 scaling for the fp8 split of
  the out-projection input).
- ao returns to [dl, tok] layout via XBAR DMA-transpose (SBUF->SBUF),
  then two DVE ops split it into ao8 + dao8 (fp8) for the 3-term
  out-projection.
- RoPE tables, probs and mask all fp16 (DVE 2x/4x perf modes).
- b0 attention is interleaved into phase-1 chunks 4-7 and the
  out-projection into b1 attention, so ACT(exp)/DMA hide under PE.
"""

import os

import numpy as np
import ml_dtypes

import concourse.bacc as bacc
import concourse.mybir as mybir
import concourse.tile as tile
from concourse.bass_utils import run_bass_kernel_spmd

B, S, D, H, HD = 2, 2048, 2048, 16, 128
T = B * S            # 4096 tokens
P = 128
NCORES = 8
HPC = H // NCORES    # 2 heads per core
DL = HPC * HD        # 256 local projection dims
DIN = D // P         # 16 contraction blocks
KP = DIN // 2        # 8 DoubleRow contraction pairs
CH = 512             # token chunk in phase 1
NCH = T // CH        # 8
QGA = 256            # q-group width in attention
NGA = S // QGA       # 8 groups per (batch, head)
NB = T // P          # 32 token blocks
SCALE = 1.0 / float(np.sqrt(HD))
EXP_BIAS = -2.0
PSC = 1.0 / 1024.0   # projection descale (64 weight x 16 residual)

f32 = mybir.dt.float32
f16 = mybir.dt.float16
f8 = mybir.dt.float8e4
AF = mybir.ActivationFunctionType
DR = mybir.MatmulPerfMode.DoubleRow

E4 = ml_dtypes.float8_e4m3

_DEBUG = bool(int(os.environ.get("K2_DEBUG", "0")))


def _build(repeat=1):
    nc = bacc.Bacc("TRN2", target_bir_lowering=False, debug=False)

    x8_d = nc.dram_tensor("x8", [P, DIN, T], f8, kind="ExternalInput")
    dx_d = nc.dram_tensor("dx16", [P, DIN, T], f8, kind="ExternalInput")
    wd = {}
    for pn in ("q", "k", "v"):
        for tn in ("a", "b", "c"):   # a=16*w8, b=w8, c=dw16
            wd[pn + tn] = nc.dram_tensor(
                f"w{pn}{tn}", [P, DIN, DL], f8, kind="ExternalInput")
    woa_d = nc.dram_tensor("woa", [P, HPC, D], f8, kind="ExternalInput")  # wo8
    wob_d = nc.dram_tensor("wob", [P, HPC, D], f8, kind="ExternalInput")  # dwo
    c2_d = nc.dram_tensor("c2", [P, T], f16, kind="ExternalInput")
    s2_d = nc.dram_tensor("s2", [P, T], f16, kind="ExternalInput")
    bq_d = nc.dram_tensor("bq2", [P, HPC], f32, kind="ExternalInput")
    bk_d = nc.dram_tensor("bk2", [P, HPC], f32, kind="ExternalInput")
    eb_d = nc.dram_tensor("ebias", [P, 1], f32, kind="ExternalInput")
    msk_d = nc.dram_tensor("msk", [P, 896], f16, kind="ExternalInput")
    y_d = nc.dram_tensor("y", [T, D], f16, kind="ExternalOutput")
    if _DEBUG:
        dbg = {n: nc.dram_tensor(f"dbg_{n}", shp, dt, kind="ExternalOutput")
               for n, shp, dt in [("qt", [P, HPC, T], f16),
                                  ("kt", [P, HPC, T], f16),
                                  ("ao8", [P, HPC, T], f8),
                                  ("dao8", [P, HPC, T], f8)]}

    with tile.TileContext(nc) as tc:
      for _rep in range(repeat):
        with tc.tile_pool(name="persist", bufs=1) as pp:
            qt = pp.tile([P, HPC, T], f16, tag="qt")
            kt = pp.tile([P, HPC, T], f16, tag="kt")
            vt = pp.tile([P, NB, HPC, 132], f16, tag="vt")
            ao8 = pp.tile([P, HPC, T], f8, tag="ao8")
            dao8 = pp.tile([P, HPC, T], f8, tag="dao8")
            c2f = pp.tile([P, T], f16, tag="c2f")
            s2f = pp.tile([P, T], f16, tag="s2f")
            msk = pp.tile([P, 896], f16, tag="msk")
            bq = pp.tile([P, HPC], f32, tag="bq")
            bk = pp.tile([P, HPC], f32, tag="bk")
            ebias = pp.tile([P, 1], f32, tag="ebias")
            w = {}
            for pn in ("q", "k", "v"):
                for tn in ("a", "b", "c"):
                    w[pn + tn] = pp.tile([P, DIN, DL], f8, tag=f"w{pn}{tn}",
                                          name=f"w{pn}{tn}")
            woa = pp.tile([P, HPC, D], f8, tag="woa")
            wob = pp.tile([P, HPC, D], f8, tag="wob")

            # constants / weights (ordered so chunk-0 needs come first)
            nc.sync.dma_start(bq[:], bq_d.ap())
            nc.sync.dma_start(bk[:], bk_d.ap())
            nc.sync.dma_start(ebias[:], eb_d.ap())
            for tn in ("a", "b", "c"):
                nc.sync.dma_start(w["q" + tn][:], wd["q" + tn].ap())
                nc.sync.dma_start(w["k" + tn][:], wd["k" + tn].ap())
                nc.sync.dma_start(w["v" + tn][:], wd["v" + tn].ap())
            nc.sync.dma_start(c2f[:], c2_d.ap())
            nc.sync.dma_start(s2f[:], s2_d.ap())
            nc.sync.dma_start(msk[:], msk_d.ap())
            nc.sync.dma_start(woa[:], woa_d.ap())
            nc.sync.dma_start(wob[:], wob_d.ap())
            nc.vector.memset(vt[:, :, :, 128:129], 0.0625)  # ones/16 column

            with tc.tile_pool(name="scoreps", bufs=3, space="PSUM") as pss, \
                 tc.tile_pool(name="avps", bufs=2, space="PSUM") as pso, \
                 tc.tile_pool(name="probsp", bufs=10) as prp, \
                 tc.tile_pool(name="aoqp", bufs=20) as aoqp, \
                 tc.tile_pool(name="aotp", bufs=8) as aotp, \
                 tc.tile_pool(name="rcpp", bufs=4) as rcpp:

                def attn_unit(b, g, h):
                    """scores+exp+AV+normalize for one (batch, q-group, head).
                    QGA q-tokens, causal k range; psum [q, hd|denom/16]."""
                    q0 = b * S + g * QGA
                    nk = (g + 1) * (QGA // P)
                    po = pso.tile([P, 2, 132], f32, tag="pso",
                                  name=f"pso{_rep}_{b}_{g}_{h}")
                    probs_l = []
                    for ki in range(0, nk, 2):
                        ps_s = pss.tile([P, 2, QGA], f32, tag="pss",
                                        name=f"pss{_rep}_{b}_{g}_{h}_{ki}")
                        for i in range(2):
                            k0 = b * S + (ki + i) * P
                            nc.tensor.matmul(ps_s[:, i],
                                             kt[:, h, k0:k0 + P],
                                             qt[:, h, q0:q0 + QGA],
                                             start=True, stop=True)
                        probs = prp.tile([P, 2, QGA], f16, tag="probs")
                        nc.scalar.activation(probs[:], ps_s[:], AF.Exp,
                                             bias=ebias[:, 0:1], scale=SCALE)
                        for i in range(2):
                            off = (ki + i) * P - g * QGA
                            if off >= 0:
                                nc.vector.tensor_mul(
                                    probs[:, i], probs[:, i],
                                    msk[:, 384 - off:640 - off])
                        probs_l.append(probs)
                    # one accumulation group at a time per psum zero-region:
                    # finish the qs=0 group over all k before opening qs=1
                    for qs in range(2):
                        for t_ in range(nk):
                            blk = b * (S // P) + t_
                            nc.tensor.matmul(
                                po[:, qs, 0:129],
                                probs_l[t_ // 2][:, t_ % 2, qs * P:(qs + 1) * P],
                                vt[:, blk, h, 0:129],
                                start=(t_ == 0), stop=(t_ == nk - 1))
                    for qs in range(2):
                        tok = q0 + qs * P
                        rcp = rcpp.tile([P, 1], f32, tag="rcp")
                        nc.vector.reciprocal(rcp[:], po[:, qs, 128:129])
                        aoq = aoqp.tile([P, P], f16, tag="aoq")
                        nc.scalar.activation(aoq[:], po[:, qs, 0:128],
                                             AF.Identity, scale=rcp[:, 0:1])
                        aot = aotp.tile([P, P], f16, tag="aot")
                        nc.sync.dma_start_transpose(aot[:], aoq[:])
                        nc.vector.tensor_copy(ao8[:, h, tok:tok + P], aot[:])
                        nc.vector.tensor_sub(dao8[:, h, tok:tok + P], aot[:],
                                             ao8[:, h, tok:tok + P])

                b0_units = [(0, g, h) for g in range(NGA) for h in range(HPC)]
                b1_units = [(1, g, h) for g in range(NGA) for h in range(HPC)]

                # ---------------- phase 1 + b0 attention ----------------
                with tc.tile_pool(name="xp", bufs=2) as xpp, \
                     tc.tile_pool(name="sbqp", bufs=4) as sbqp, \
                     tc.tile_pool(name="p1qk", bufs=2, space="PSUM") as pqk, \
                     tc.tile_pool(name="p1v", bufs=1, space="PSUM") as pv:
                    for ch in range(NCH):
                        t0 = ch * CH
                        x8c = xpp.tile([P, DIN, CH], f8, tag="x8c",
                                       name=f"x8c{_rep}_{ch}")
                        dxc = xpp.tile([P, DIN, CH], f8, tag="dxc",
                                       name=f"dxc{_rep}_{ch}")
                        nc.sync.dma_start(x8c[:], x8_d.ap()[:, :, t0:t0 + CH])
                        nc.sync.dma_start(dxc[:], dx_d.ap()[:, :, t0:t0 + CH])

                        # Q^T / K^T: out [128 dims(head m), CH tok]
                        for pn, pool, bias_t, dst in (
                                ("q", pqk, bq, qt), ("k", pqk, bk, kt)):
                            for m in range(HPC):
                                ps = pool.tile([P, CH], f32, tag="ps",
                                               name=f"ps{pn}{_rep}_{ch}_{m}")
                                ms = slice(m * P, (m + 1) * P)
                                terms = ((w[pn + "a"], x8c), (w[pn + "b"], dxc),
                                         (w[pn + "c"], x8c))
                                for ti, (lt, rt) in enumerate(terms):
                                    for j in range(KP):
                                        nc.tensor.matmul(
                                            ps[:],
                                            lt[:, 2 * j:2 * j + 2, ms],
                                            rt[:, 2 * j:2 * j + 2, :],
                                            start=(ti == 0 and j == 0),
                                            stop=(ti == 2 and j == KP - 1),
                                            perf_mode=DR)
                                sbq = sbqp.tile([P, CH], f16, tag="sbq")
                                nc.scalar.activation(sbq[:], ps[:], AF.Identity,
                                                     bias=bias_t[:, m:m + 1],
                                                     scale=PSC)
                                # RoPE: dst = sbq*c2 + halfswap(sbq)*s2
                                sw = sbqp.tile([P, CH], f16, tag="sw")
                                HH = P // 2
                                nc.vector.tensor_copy(sw[0:HH, :], sbq[HH:P, :])
                                nc.vector.tensor_copy(sw[HH:P, :], sbq[0:HH, :])
                                dslc = dst[:, m, t0:t0 + CH]
                                nc.vector.tensor_mul(dslc, sbq[:],
                                                     c2f[:, t0:t0 + CH])
                                nc.vector.tensor_mul(sw[:], sw[:],
                                                     s2f[:, t0:t0 + CH])
                                nc.vector.tensor_add(dslc, dslc, sw[:])

                        # V: out [128 tok, 256 dl] per token-subtile
                        for half in range(2):
                            psv = pv.tile([P, 2, DL], f32, tag="psv",
                                          name=f"psv{_rep}_{ch}_{half}")
                            for s_ in range(2):
                                toff = half * 256 + s_ * P
                                tsl = slice(toff, toff + P)
                                terms = ((x8c, w["va"]), (dxc, w["vb"]),
                                         (x8c, w["vc"]))
                                for ti, (lt, rt) in enumerate(terms):
                                    for j in range(KP):
                                        nc.tensor.matmul(
                                            psv[:, s_],
                                            lt[:, 2 * j:2 * j + 2, tsl],
                                            rt[:, 2 * j:2 * j + 2, :],
                                            start=(ti == 0 and j == 0),
                                            stop=(ti == 2 and j == KP - 1),
                                            perf_mode=DR)
                            for s_ in range(2):
                                blk = (t0 + half * 256 + s_ * P) // P
                                for hh in range(HPC):
                                    nc.scalar.activation(
                                        vt[:, blk, hh, 0:P],
                                        psv[:, s_, hh * P:(hh + 1) * P],
                                        AF.Identity, scale=PSC)

                        if ch >= NCH // 2:
                            k4 = 4 * (ch - NCH // 2)
                            for unit in b0_units[k4:k4 + 4]:
                                attn_unit(*unit)

                # ---------------- b1 attention + out-projection ----------------
                with tc.tile_pool(name="yp", bufs=6) as yp, \
                     tc.tile_pool(name="p3ps", bufs=2, space="PSUM") as pyps:

                    def out_block(tb, tail=False):
                        y_sb = yp.tile([P, D], f16, tag="ysb",
                                       name=f"ysb{_rep}_{tb}")
                        tsl = slice(tb * P, (tb + 1) * P)
                        for dc in range(D // 512):
                            ps_y = pyps.tile([P, 512], f32, tag="psy",
                                             name=f"psy{_rep}_{tb}_{dc}")
                            dsl = slice(dc * 512, (dc + 1) * 512)
                            nc.tensor.matmul(ps_y[:], ao8[:, :, tsl],
                                             woa[:, :, dsl],
                                             start=True, stop=False,
                                             perf_mode=DR)
                            nc.tensor.matmul(ps_y[:], dao8[:, :, tsl],
                                             woa[:, :, dsl],
                                             start=False, stop=False,
                                             perf_mode=DR)
                            nc.tensor.matmul(ps_y[:], ao8[:, :, tsl],
                                             wob[:, :, dsl],
                                             start=False, stop=True,
                                             perf_mode=DR)
                            eng = (nc.vector if (tb + dc) % 2 == 0
                                   else nc.gpsimd)
                            eng.tensor_scalar_mul(y_sb[:, dsl], ps_y[:], PSC)
                        nc.sync.dma_start(y_d.ap()[tb * P:(tb + 1) * P, :],
                                          y_sb[:])

                    ready = list(range(S // P))  # b0 token blocks
                    done = 0
                    for ui, (b, g, h) in enumerate(b1_units):
                        attn_unit(b, g, h)
                        if h == HPC - 1:
                            ready += [S // P + 2 * g, S // P + 2 * g + 1]
                        while done < min(len(ready), 2 * (ui + 1)):
                            out_block(ready[done])
                            done += 1
                    while done < len(ready):
                        out_block(ready[done])
                        done += 1

            if _DEBUG:
                for n, t in [("qt", qt), ("kt", kt), ("ao8", ao8),
                             ("dao8", dao8)]:
                    nc.sync.dma_start(dbg[n].ap(), t[:])

    nc.compile()
    return nc


_NC = None


def _get_nc():
    global _NC
    if _NC is None:
        _NC = _build()
    return _NC


def _q8(a):
    return np.asarray(a, np.float32).astype(E4)


def _f(a):
    return np.asarray(a, np.float32)


def _prep_inputs(x, wq, bq, wk, bk, wv, bv, wo, bo, freqs_cos, freqs_sin):
    """Host-side marshalling: layout transforms + fp8 residual splits."""
    f = np.float32
    xT = np.asarray(x, f).reshape(T, D).T                     # [D, T]
    xp = np.ascontiguousarray(xT.reshape(DIN, P, T).transpose(1, 0, 2))
    x8 = _q8(xp)
    dx16 = _q8(16.0 * (xp - _f(x8)))

    # per-head row permutation: [evens, odds] so rope pairs sit in halves
    perm1 = np.concatenate([np.arange(0, HD, 2), np.arange(1, HD, 2)])
    perm = np.concatenate([h * HD + perm1 for h in range(HPC)])

    cosT = np.asarray(freqs_cos, f).T                         # [64, S]
    sinT = np.asarray(freqs_sin, f).T
    c2 = np.tile(np.concatenate([cosT, cosT], 0), (1, B)).astype(np.float16)
    s2 = np.tile(np.concatenate([-sinT, sinT], 0), (1, B)).astype(np.float16)

    jj, kk = np.meshgrid(np.arange(896), np.arange(P), indexing="xy")
    mskv = (jj - 384 >= kk).astype(np.float16)

    ebv = np.full((P, 1), EXP_BIAS, f)

    def wsplit(wfull, c, permute):
        wc = np.asarray(wfull, f)[c * DL:(c + 1) * DL]        # [DL, D]
        if permute:
            wc = wc[perm]
        wt = np.ascontiguousarray(
            wc.T.reshape(DIN, P, DL).transpose(1, 0, 2))      # [P, DIN, DL]
        w64 = 64.0 * wt
        w8 = _q8(w64)
        wa = _q8(16.0 * _f(w8))
        wc8 = _q8(16.0 * (w64 - _f(w8)))
        return wa, w8, wc8

    wo_f = np.asarray(wo, f)
    in_maps = []
    for c in range(NCORES):
        m = {"x8": x8, "dx16": dx16, "c2": c2, "s2": s2, "msk": mskv,
             "ebias": ebv}
        for pn, wfull, permute in (("q", wq, True), ("k", wk, True),
                                   ("v", wv, False)):
            wa, wb, wc8 = wsplit(wfull, c, permute)
            m[f"w{pn}a"], m[f"w{pn}b"], m[f"w{pn}c"] = wa, wb, wc8
        woc = wo_f[:, c * DL:(c + 1) * DL]                    # [D, DL]
        wot = np.ascontiguousarray(
            woc.T.reshape(HPC, P, D).transpose(1, 0, 2))      # [P, HPC, D]
        wo64 = 64.0 * wot
        wo8 = _q8(wo64)
        m["woa"] = wo8
        m["wob"] = _q8(wo64 - _f(wo8))
        bqc = np.asarray(bq, f)[c * DL:(c + 1) * DL][perm]
        bkc = np.asarray(bk, f)[c * DL:(c + 1) * DL][perm]
        m["bq2"] = np.ascontiguousarray(bqc.reshape(HPC, P).T)
        m["bk2"] = np.ascontiguousarray(bkc.reshape(HPC, P).T)
        in_maps.append(m)
    return in_maps


def _run(in_maps, trace=False):
    nc = _get_nc()
    return run_bass_kernel_spmd(nc, in_maps, core_ids=list(range(NCORES)),
                                trace=trace)


def kernel(**inputs):
    in_maps = _prep_inputs(**inputs)
    res = _run(in_maps)
    y = np.zeros((T, D), np.float32)
    for c in range(NCORES):
        y += res.results[c]["y"].astype(np.float32)
    bv = np.asarray(inputs["bv"], np.float32)
    bo = np.asarray(inputs["bo"], np.float32)
    wo = np.asarray(inputs["wo"], np.float32)
    y += (bo + bv @ wo.T)[None, :]
    return y.reshape(B, S, D)
